# revision 11
# baseline (speedup 1.0000x reference)
"""AttnBlock (GroupNorm + 1x1-conv QKV + NxN attention + proj + residual) on 8 NeuronCores.

Sharding: data-parallel over batch (4 samples) x 2-way sequence-parallel over
query rows. Each core gets one sample's full (C,N) activation with its query
half permuted to columns 0:2048, computes GroupNorm stats, normalizes, runs
scores/softmax/AV in a j-transposed layout (so no on-chip transposes are
needed anywhere), and emits its 2048 output columns transposed (positions on
partitions) so the softmax denominator can be applied as a per-partition
scalar.

All heavy matmuls run in bf16 with fp32 PSUM accumulation; statistics,
softmax denominators and the residual path stay in fp32.
"""

import numpy as np
import ml_dtypes
from contextlib import ExitStack

import concourse.bass as bass
import concourse.bacc as bacc
import concourse.mybir as mybir
import concourse.tile as tile
from concourse.tile_rust import add_dep_helper
from concourse.bass_utils import run_bass_kernel_spmd

F32 = mybir.dt.float32
BF16 = mybir.dt.bfloat16
AF = mybir.ActivationFunctionType
ALU = mybir.AluOpType

C = 512          # channels
NSEQ = 4096      # sequence length (H*W)
NQ = 2048        # query rows per core (sequence-parallel 2-way)
P = 128          # partitions
NCH = C // P     # 4 channel chunks
NJ = NSEQ // P   # 32 key-position chunks
NI = NQ // 512   # 4 query chunks of 512
EPS = 1e-6
SCALE = float(C) ** -0.5
CNT_INV = 1.0 / (16 * NSEQ)   # elements per group (16 ch x 4096 positions)


def build_nc():
    nc = bacc.Bacc("TRN2", target_bir_lowering=False, debug=False)

    x_d = nc.dram_tensor("x", [C, NSEQ], F32, kind="ExternalInput")
    wqT_d = nc.dram_tensor("wqT", [C, C], BF16, kind="ExternalInput")
    wkT_d = nc.dram_tensor("wkT", [C, C], BF16, kind="ExternalInput")
    wvT_d = nc.dram_tensor("wvT", [C, C], BF16, kind="ExternalInput")
    wpT_d = nc.dram_tensor("wpT", [C, C], BF16, kind="ExternalInput")
    # packed per-channel vectors: cols 0=bq 1=bk 2=gn_w 3=gn_b
    bpk_d = nc.dram_tensor("bpk", [C, 4], F32, kind="ExternalInput")
    bvr_d = nc.dram_tensor("bvr", [1, C], BF16, kind="ExternalInput")
    g_d = nc.dram_tensor("gmat", [P, P], F32, kind="ExternalInput")
    xpbT_d = nc.dram_tensor("xpbT", [NQ, C], F32, kind="ExternalInput")
    out_d = nc.dram_tensor("outT", [NQ, C], F32, kind="ExternalOutput")

    x_ch = x_d.rearrange("(c p) n -> c p n", p=P)
    bpk_ch = bpk_d.rearrange("(c p) k -> c p k", p=P)

    with tile.TileContext(nc) as tc, ExitStack() as ctx:
        psum = ctx.enter_context(tc.tile_pool(name="psum", bufs=4, space="PSUM"))
        consts = ctx.enter_context(tc.tile_pool(name="consts", bufs=1))
        wpool = ctx.enter_context(tc.tile_pool(name="wpool", bufs=1))
        hp = ctx.enter_context(tc.tile_pool(name="hp", bufs=1))

        # ---- constants ----
        g_sb = consts.tile([P, P], F32, tag="g")
        nc.sync.dma_start(g_sb[:], g_d[:])
        bpk_sb = []
        for ci in range(NCH):
            t = consts.tile([P, 4], F32, tag=f"bpk{ci}", name=f"bpk{ci}")
            nc.sync.dma_start(t[:], bpk_ch[ci])
            bpk_sb.append(t)
        bvr_sb = consts.tile([1, C], BF16, tag="bvr")
        nc.sync.dma_start(bvr_sb[:], bvr_d[:])
        ones_row = consts.tile([1, P], BF16, tag="ones1")
        nc.vector.memset(ones_row[:], 1.0)
        ones_col = consts.tile([P, 1], F32, tag="ones2")
        nc.vector.memset(ones_col[:], 1.0)

        wt = {}
        for wn, wd in (("q", wqT_d), ("k", wkT_d), ("v", wvT_d), ("p", wpT_d)):
            wt[wn] = []
            for ci in range(NCH):
                t = wpool.tile([P, C], BF16, tag=f"w{wn}{ci}", name=f"w{wn}{ci}")
                nc.sync.dma_start(t[:], wd[ci * P:(ci + 1) * P, :])
                wt[wn].append(t)

        # ---- phase A: stream x in 512-col blocks, per-block stats so the
        # stats pipeline trails the DMA instead of waiting for all of x ----
        h_sb = []
        for ci in range(NCH):
            t = hp.tile([P, NSEQ], BF16, tag=f"h{ci}", name=f"h{ci}")
            h_sb.append(t)
        NB = NSEQ // 512  # 8 blocks per chunk
        with tc.tile_pool(name="xsp", bufs=1) as xsp:
            xs_t, sp_t, ssp_t = [], [], []
            for ci in range(NCH):
                xs = xsp.tile([P, NSEQ], F32, tag=f"xs{ci}", bufs=1,
                              name=f"xs{ci}")
                xs_t.append(xs)
                spart = consts.tile([P, NB], F32, tag=f"sp{ci}", name=f"sp{ci}")
                sp_t.append(spart)
                sspart = consts.tile([P, NB], F32, tag=f"ssp{ci}",
                                     name=f"ssp{ci}")
                ssp_t.append(sspart)
            for ci in range(NCH):
                for b in range(NB):
                    sl = slice(b * 512, (b + 1) * 512)
                    nc.sync.dma_start(xs_t[ci][:, sl], x_ch[ci][:, sl])
                    nc.vector.tensor_reduce(sp_t[ci][:, b:b + 1],
                                            xs_t[ci][:, sl],
                                            axis=mybir.AxisListType.X,
                                            op=ALU.add)
                    sq = xsp.tile([P, 512], BF16, tag="sq", bufs=2,
                                  name=f"sq{ci}_{b}")
                    nc.scalar.activation(sq[:], xs_t[ci][:, sl], AF.Square,
                                         accum_out=ssp_t[ci][:, b:b + 1])

            # ---- group stats -> per-channel affine A, B (batched: one
            # column per chunk; st8 columns are (s0,ss0,s1,ss1,...)) ----
            st8 = consts.tile([P, 2 * NCH], F32, tag="st8")
            for ci in range(NCH):
                nc.vector.tensor_reduce(st8[:, 2 * ci:2 * ci + 1], sp_t[ci][:],
                                        axis=mybir.AxisListType.X, op=ALU.add)
                nc.vector.tensor_reduce(st8[:, 2 * ci + 1:2 * ci + 2],
                                        ssp_t[ci][:],
                                        axis=mybir.AxisListType.X, op=ALU.add)
            gps = psum.tile([P, 2 * NCH], F32, tag="mm", name="gps")
            nc.tensor.matmul(gps[:], lhsT=g_sb[:], rhs=st8[:], start=True,
                             stop=True)
            gnw8 = consts.tile([P, NCH], F32, tag="gnw8")
            gnb8 = consts.tile([P, NCH], F32, tag="gnb8")
            for ci in range(NCH):
                nc.vector.tensor_copy(gnw8[:, ci:ci + 1], bpk_sb[ci][:, 2:3])
                nc.vector.tensor_copy(gnb8[:, ci:ci + 1], bpk_sb[ci][:, 3:4])
            mean = consts.tile([P, NCH], F32, tag="mean")
            nc.vector.tensor_scalar_mul(mean[:], gps[:, 0:2 * NCH:2], CNT_INV)
            ex2 = consts.tile([P, NCH], F32, tag="ex2")
            nc.vector.tensor_scalar_mul(ex2[:], gps[:, 1:2 * NCH:2], CNT_INV)
            msq = consts.tile([P, NCH], F32, tag="msq")
            nc.vector.tensor_mul(msq[:], mean[:], mean[:])
            vpe = consts.tile([P, NCH], F32, tag="vpe")
            # (ex2 + EPS) - mean^2
            nc.vector.scalar_tensor_tensor(vpe[:], in0=ex2[:], scalar=EPS,
                                           in1=msq[:], op0=ALU.add,
                                           op1=ALU.subtract)
            rvar = consts.tile([P, NCH], F32, tag="rvar")
            nc.vector.reciprocal(rvar[:], vpe[:])
            rstd = consts.tile([P, NCH], F32, tag="rstd")
            nc.scalar.activation(rstd[:], rvar[:], AF.Sqrt)
            Aall = consts.tile([P, NCH], F32, tag="Aall")
            nc.vector.tensor_mul(Aall[:], rstd[:], gnw8[:])
            nmA = consts.tile([P, NCH], F32, tag="nmA")
            # (mean * -1) * A
            nc.vector.scalar_tensor_tensor(nmA[:], in0=mean[:], scalar=-1.0,
                                           in1=Aall[:], op0=ALU.mult,
                                           op1=ALU.mult)
            Ball = consts.tile([P, NCH], F32, tag="Ball")
            nc.vector.tensor_add(Ball[:], nmA[:], gnb8[:])
            A_t = [Aall[:, ci:ci + 1] for ci in range(NCH)]
            B_t = [Ball[:, ci:ci + 1] for ci in range(NCH)]

            # ---- h = A*x + B, emitted column-block-major so projections can
            # start as soon as the first column block of every chunk exists
            for jt in range(NSEQ // 512):
                for ci in range(NCH):
                    sl = slice(jt * 512, (jt + 1) * 512)
                    nc.vector.tensor_scalar(h_sb[ci][:, sl], xs_t[ci][:, sl],
                                            A_t[ci][:], B_t[ci][:],
                                            op0=ALU.mult, op1=ALU.add)

        # streaming pool is closed; attention-phase pools may now reuse its
        # SBUF range
        kqp = ctx.enter_context(tc.tile_pool(name="kqp", bufs=1))
        attp = ctx.enter_context(tc.tile_pool(name="attp", bufs=1))
        outp = ctx.enter_context(tc.tile_pool(name="outp", bufs=1))

        # ---- projections ----
        # vT[j] : 32 tiles of [128 (j), 512 (c)], bias row via K=1 matmul
        vt_sb = []
        for jt in range(NJ):
            ps = psum.tile([P, C], F32, tag="mm", name=f"vps{jt}")
            for ci in range(NCH):
                nc.tensor.matmul(ps[:], lhsT=h_sb[ci][:, jt * P:(jt + 1) * P],
                                 rhs=wt["v"][ci][:], start=(ci == 0), stop=False)
            nc.tensor.matmul(ps[:], lhsT=ones_row[:], rhs=bvr_sb[:],
                             start=False, stop=True)
            vtt = kqp.tile([P, C], BF16, tag="vt", bufs=NJ, name=f"vt{jt}")
            nc.vector.tensor_copy(vtt[:], ps[:])
            vt_sb.append(vtt)
        # k[co, j] : 4 chunks of [128, 4096]
        k_sb = []
        for co in range(NCH):
            t = kqp.tile([P, NSEQ], BF16, tag=f"k{co}", name=f"k{co}")
            k_sb.append(t)
        for co in range(NCH):
            for jt in range(NSEQ // 512):
                ps = psum.tile([P, 512], F32, tag="mm", name=f"kps{co}_{jt}")
                for ci in range(NCH):
                    nc.tensor.matmul(ps[:], lhsT=wt["k"][ci][:, co * P:(co + 1) * P],
                                     rhs=h_sb[ci][:, jt * 512:(jt + 1) * 512],
                                     start=(ci == 0), stop=(ci == NCH - 1))
                nc.scalar.activation(k_sb[co][:, jt * 512:(jt + 1) * 512], ps[:],
                                     AF.Identity, bias=bpk_sb[co][:, 1:2])
        # q[co, i] : 4 chunks of [128, 2048] (own query half = cols 0:2048)
        q_sb = []
        for co in range(NCH):
            t = kqp.tile([P, NQ], BF16, tag=f"q{co}", name=f"q{co}")
            q_sb.append(t)
        for it in range(NQ // 512):
            for co in range(NCH):
                ps = psum.tile([P, 512], F32, tag="mm", name=f"qps{co}_{it}")
                for ci in range(NCH):
                    nc.tensor.matmul(ps[:], lhsT=wt["q"][ci][:, co * P:(co + 1) * P],
                                     rhs=h_sb[ci][:, it * 512:(it + 1) * 512],
                                     start=(ci == 0), stop=(ci == NCH - 1))
                nc.scalar.activation(q_sb[co][:, it * 512:(it + 1) * 512], ps[:],
                                     AF.Identity, bias=bpk_sb[co][:, 0:1])

        # ---- attention + fused output projection ----
        for ic in range(NI):
            accs = [psum.tile([P, 512], F32, tag="acc", name=f"acc{ic}_{c}")
                    for c in range(NCH)]
            eacc_prev = None
            for jt in range(NJ):
                ps = psum.tile([P, 512], F32, tag="mm", name=f"sps{ic}_{jt}")
                for ci in range(NCH):
                    nc.tensor.matmul(ps[:], lhsT=k_sb[ci][:, jt * P:(jt + 1) * P],
                                     rhs=q_sb[ci][:, ic * 512:(ic + 1) * 512],
                                     start=(ci == 0), stop=(ci == NCH - 1))
                et = attp.tile([P, 512], BF16, tag="et", bufs=3,
                               name=f"et{ic}_{jt}")
                nc.scalar.activation(et[:], ps[:], AF.Exp, scale=SCALE)
                eacc = attp.tile([P, 512], F32, tag="ea", bufs=2,
                                 name=f"ea{ic}_{jt}")
                if jt == 0:
                    nc.vector.tensor_copy(eacc[:], et[:])
                else:
                    nc.vector.tensor_add(eacc[:], eacc_prev[:], et[:])
                eacc_prev = eacc
                for c in range(NCH):
                    nc.tensor.matmul(accs[c][:], lhsT=vt_sb[jt][:, c * P:(c + 1) * P],
                                     rhs=et[:], start=(jt == 0), stop=(jt == NJ - 1))
            h2c = []
            for c in range(NCH):
                h2t = attp.tile([P, 512], BF16, tag="h2", bufs=2 * NCH,
                                name=f"h2_{ic}_{c}")
                nc.scalar.copy(h2t[:], accs[c][:])
                h2c.append(h2t)
            rcs = []
            gate_inst = None
            for iq in range(4):
                dps = psum.tile([P, 1], F32, tag="mm", name=f"dps{ic}_{iq}")
                mm_i = nc.tensor.matmul(dps[:],
                                        lhsT=eacc_prev[:, iq * P:(iq + 1) * P],
                                        rhs=ones_col[:], start=True, stop=True)
                if iq == 0:
                    gate_inst = mm_i
                rc = consts.tile([P, 1], F32, tag=f"rc{ic * 4 + iq}",
                                 name=f"rc{ic * 4 + iq}")
                nc.vector.reciprocal(rc[:], dps[:])
                rcs.append(rc)
            # output projection for this i-chunk (transposed) + residual
            for iq in range(4):
                t_i = ic * 4 + iq
                pps = psum.tile([P, C], F32, tag="mm", name=f"pps{t_i}")
                for c in range(NCH):
                    nc.tensor.matmul(pps[:], lhsT=h2c[c][:, iq * P:(iq + 1) * P],
                                     rhs=wt["p"][c][:], start=(c == 0),
                                     stop=(c == NCH - 1))
                xt = outp.tile([P, C], F32, tag="xr", bufs=3, name=f"xt{t_i}")
                xt_dma = nc.sync.dma_start(xt[:], xpbT_d[t_i * P:(t_i + 1) * P, :])
                # keep the residual loads out of the phase-A DMA window: only
                # issue them once this i-chunk's attention is winding down
                add_dep_helper(xt_dma.ins, gate_inst.ins, sync=True,
                               reason="delay residual load")
                ot = outp.tile([P, C], F32, tag="ot", bufs=3, name=f"ot{t_i}")
                nc.vector.scalar_tensor_tensor(ot[:], in0=pps[:],
                                               scalar=rcs[iq][:], in1=xt[:],
                                               op0=ALU.mult, op1=ALU.add)
                nc.sync.dma_start(out_d[t_i * P:(t_i + 1) * P, :], ot[:])

    nc.compile()
    if not nc.is_finalized():
        nc.finalize()
    return nc


_NC_CACHE = None


def _get_nc():
    global _NC_CACHE
    if _NC_CACHE is None:
        _NC_CACHE = build_nc()
    return _NC_CACHE


def make_in_maps(x, gn_w, gn_b, wq, bq, wk, bk, wv, bv, wp, bp):
    bf = ml_dtypes.bfloat16
    x = np.asarray(x, np.float32)
    B = x.shape[0]
    shared = {
        "wqT": np.ascontiguousarray(np.asarray(wq, np.float32).T).astype(bf),
        "wkT": np.ascontiguousarray(np.asarray(wk, np.float32).T).astype(bf),
        "wvT": np.ascontiguousarray(np.asarray(wv, np.float32).T).astype(bf),
        "wpT": np.ascontiguousarray(np.asarray(wp, np.float32).T).astype(bf),
        "bpk": np.ascontiguousarray(
            np.stack([bq, bk, gn_w, gn_b], axis=1).astype(np.float32)),
        "bvr": np.asarray(bv, np.float32).reshape(1, C).astype(bf),
        "gmat": np.kron(np.eye(8, dtype=np.float32),
                        np.ones((16, 16), np.float32)),
    }
    in_maps = []
    for core in range(2 * B):
        b, h = divmod(core, 2)
        xb2 = x[b].reshape(C, NSEQ)
        own = xb2[:, h * NQ:(h + 1) * NQ]
        other = xb2[:, (1 - h) * NQ:(2 - h) * NQ]
        m = dict(shared)
        m["x"] = np.ascontiguousarray(np.concatenate([own, other], axis=1))
        m["xpbT"] = np.ascontiguousarray(own.T + np.asarray(bp, np.float32)[None, :])
        in_maps.append(m)
    return in_maps


def kernel(x, gn_w, gn_b, wq, bq, wk, bk, wv, bv, wp, bp, _run_kwargs=None):
    x = np.asarray(x)
    B, C_, H, W = x.shape
    nc = _get_nc()
    in_maps = make_in_maps(x, gn_w, gn_b, wq, bq, wk, bk, wv, bv, wp, bp)
    res = run_bass_kernel_spmd(nc, in_maps, list(range(2 * B)),
                               **(_run_kwargs or {}))
    out = np.empty((B, C, NSEQ), np.float32)
    for core in range(2 * B):
        b, h = divmod(core, 2)
        out[b][:, h * NQ:(h + 1) * NQ] = res.results[core]["outT"].T
    out = out.reshape(B, C, H, W).astype(x.dtype, copy=False)
    kernel.last_results = res
    return out


# revision 12
# speedup vs baseline: 1.0674x; 1.0674x over previous
"""AttnBlock (GroupNorm + 1x1-conv QKV + NxN attention + proj + residual) on 8 NeuronCores.

Sharding: data-parallel over batch (4 samples) x 2-way sequence-parallel over
query rows. Each core gets one sample's full (C,N) activation with its query
half permuted to columns 0:2048, computes GroupNorm stats, normalizes, runs
scores/softmax/AV in a j-transposed layout (so no on-chip transposes are
needed anywhere), and emits its 2048 output columns transposed (positions on
partitions) so the softmax denominator can be applied as a per-partition
scalar.

All heavy matmuls run in bf16 with fp32 PSUM accumulation; statistics,
softmax denominators and the residual path stay in fp32.
"""

import numpy as np
import ml_dtypes
from contextlib import ExitStack

import concourse.bass as bass
import concourse.bacc as bacc
import concourse.mybir as mybir
import concourse.tile as tile
from concourse.tile_rust import add_dep_helper
from concourse.bass_utils import run_bass_kernel_spmd

F32 = mybir.dt.float32
BF16 = mybir.dt.bfloat16
AF = mybir.ActivationFunctionType
ALU = mybir.AluOpType

C = 512          # channels
NSEQ = 4096      # sequence length (H*W)
NQ = 2048        # query rows per core (sequence-parallel 2-way)
P = 128          # partitions
NCH = C // P     # 4 channel chunks
NJ = NSEQ // P   # 32 key-position chunks
NI = NQ // 512   # 4 query chunks of 512
EPS = 1e-6
SCALE = float(C) ** -0.5
CNT_INV = 1.0 / (16 * NSEQ)   # elements per group (16 ch x 4096 positions)


def build_nc(with_vbias=True):
    nc = bacc.Bacc("TRN2", target_bir_lowering=False, debug=False)

    x_d = nc.dram_tensor("x", [C, NSEQ], BF16, kind="ExternalInput")
    wqT_d = nc.dram_tensor("wqT", [C, C], BF16, kind="ExternalInput")
    wkT_d = nc.dram_tensor("wkT", [C, C], BF16, kind="ExternalInput")
    wvT_d = nc.dram_tensor("wvT", [C, C], BF16, kind="ExternalInput")
    wpT_d = nc.dram_tensor("wpT", [C, C], BF16, kind="ExternalInput")
    # packed per-channel vectors: cols 0=bq 1=bk 2=gn_w 3=gn_b
    bpk_d = nc.dram_tensor("bpk", [C, 4], F32, kind="ExternalInput")
    bvr_d = nc.dram_tensor("bvr", [1, C], BF16, kind="ExternalInput")
    g_d = nc.dram_tensor("gmat", [P, P], F32, kind="ExternalInput")
    xpbT_d = nc.dram_tensor("xpbT", [NQ, C], F32, kind="ExternalInput")
    out_d = nc.dram_tensor("outT", [NQ, C], F32, kind="ExternalOutput")

    x_ch = x_d.rearrange("(c p) n -> c p n", p=P)
    bpk_ch = bpk_d.rearrange("(c p) k -> c p k", p=P)

    with tile.TileContext(nc) as tc, ExitStack() as ctx:
        psum = ctx.enter_context(tc.tile_pool(name="psum", bufs=4, space="PSUM"))
        consts = ctx.enter_context(tc.tile_pool(name="consts", bufs=1))
        wpool = ctx.enter_context(tc.tile_pool(name="wpool", bufs=1))
        hp = ctx.enter_context(tc.tile_pool(name="hp", bufs=1))

        # ---- constants ----
        g_sb = consts.tile([P, P], F32, tag="g")
        nc.sync.dma_start(g_sb[:], g_d[:])
        bpk_sb = []
        for ci in range(NCH):
            t = consts.tile([P, 4], F32, tag=f"bpk{ci}", name=f"bpk{ci}")
            nc.sync.dma_start(t[:], bpk_ch[ci])
            bpk_sb.append(t)
        bvr_sb = consts.tile([1, C], BF16, tag="bvr")
        nc.sync.dma_start(bvr_sb[:], bvr_d[:])
        ones_row = consts.tile([1, P], BF16, tag="ones1")
        nc.vector.memset(ones_row[:], 1.0)
        ones_col = consts.tile([P, 1], F32, tag="ones2")
        nc.vector.memset(ones_col[:], 1.0)

        wt = {}
        for wn, wd in (("q", wqT_d), ("k", wkT_d), ("v", wvT_d), ("p", wpT_d)):
            wt[wn] = []
            for ci in range(NCH):
                t = wpool.tile([P, C], BF16, tag=f"w{wn}{ci}", name=f"w{wn}{ci}")
                nc.sync.dma_start(t[:], wd[ci * P:(ci + 1) * P, :])
                wt[wn].append(t)

        # ---- phase A: stream x (bf16), per-chunk stats ----
        h_sb = []
        for ci in range(NCH):
            t = hp.tile([P, NSEQ], BF16, tag=f"h{ci}", name=f"h{ci}")
            h_sb.append(t)
        with tc.tile_pool(name="xsp", bufs=1) as xsp:
            xs_t, sp_t, ssp_t = [], [], []
            for ci in range(NCH):
                xs = xsp.tile([P, NSEQ], BF16, tag=f"xs{ci}", bufs=1,
                              name=f"xs{ci}")
                nc.sync.dma_start(xs[:], x_ch[ci])
                xs_t.append(xs)
                st = consts.tile([P, 1], F32, tag=f"s{ci}", name=f"s{ci}")
                nc.vector.tensor_reduce(st[:], xs[:], axis=mybir.AxisListType.X,
                                        op=ALU.add)
                sp_t.append(st)
                sq = xsp.tile([P, NSEQ], BF16, tag="sq", bufs=1, name=f"sq{ci}")
                sst = consts.tile([P, 1], F32, tag=f"ss{ci}", name=f"ss{ci}")
                nc.scalar.activation(sq[:], xs[:], AF.Square, accum_out=sst[:])
                ssp_t.append(sst)

            # ---- group stats -> per-channel affine A, B (batched; st8
            # columns are (s0,ss0,s1,ss1,...)) ----
            st8 = consts.tile([P, 2 * NCH], F32, tag="st8")
            for ci in range(NCH):
                nc.vector.tensor_copy(st8[:, 2 * ci:2 * ci + 1], sp_t[ci][:])
                nc.vector.tensor_copy(st8[:, 2 * ci + 1:2 * ci + 2],
                                      ssp_t[ci][:])
            gps = psum.tile([P, 2 * NCH], F32, tag="mm", name="gps")
            nc.tensor.matmul(gps[:], lhsT=g_sb[:], rhs=st8[:], start=True,
                             stop=True)
            gnw8 = consts.tile([P, NCH], F32, tag="gnw8")
            gnb8 = consts.tile([P, NCH], F32, tag="gnb8")
            for ci in range(NCH):
                nc.vector.tensor_copy(gnw8[:, ci:ci + 1], bpk_sb[ci][:, 2:3])
                nc.vector.tensor_copy(gnb8[:, ci:ci + 1], bpk_sb[ci][:, 3:4])
            mean = consts.tile([P, NCH], F32, tag="mean")
            nc.vector.tensor_scalar_mul(mean[:], gps[:, 0:2 * NCH:2], CNT_INV)
            ex2 = consts.tile([P, NCH], F32, tag="ex2")
            nc.vector.tensor_scalar_mul(ex2[:], gps[:, 1:2 * NCH:2], CNT_INV)
            msq = consts.tile([P, NCH], F32, tag="msq")
            nc.vector.tensor_mul(msq[:], mean[:], mean[:])
            vpe = consts.tile([P, NCH], F32, tag="vpe")
            # (ex2 + EPS) - mean^2
            nc.vector.scalar_tensor_tensor(vpe[:], in0=ex2[:], scalar=EPS,
                                           in1=msq[:], op0=ALU.add,
                                           op1=ALU.subtract)
            rvar = consts.tile([P, NCH], F32, tag="rvar")
            nc.vector.reciprocal(rvar[:], vpe[:])
            rstd = consts.tile([P, NCH], F32, tag="rstd")
            nc.scalar.activation(rstd[:], rvar[:], AF.Sqrt)
            Aall = consts.tile([P, NCH], F32, tag="Aall")
            nc.vector.tensor_mul(Aall[:], rstd[:], gnw8[:])
            nmA = consts.tile([P, NCH], F32, tag="nmA")
            # (mean * -1) * A
            nc.vector.scalar_tensor_tensor(nmA[:], in0=mean[:], scalar=-1.0,
                                           in1=Aall[:], op0=ALU.mult,
                                           op1=ALU.mult)
            Ball = consts.tile([P, NCH], F32, tag="Ball")
            nc.vector.tensor_add(Ball[:], nmA[:], gnb8[:])
            A_t = [Aall[:, ci:ci + 1] for ci in range(NCH)]
            B_t = [Ball[:, ci:ci + 1] for ci in range(NCH)]

            # ---- h = A*x + B, column-block-major, split across DVE/ACT ----
            for jt in range(NSEQ // 512):
                for ci in range(NCH):
                    sl = slice(jt * 512, (jt + 1) * 512)
                    if (jt * NCH + ci) % 2 == 0:
                        nc.vector.tensor_scalar(h_sb[ci][:, sl],
                                                xs_t[ci][:, sl],
                                                A_t[ci], B_t[ci],
                                                op0=ALU.mult, op1=ALU.add)
                    else:
                        nc.scalar.activation(h_sb[ci][:, sl], xs_t[ci][:, sl],
                                             AF.Identity, bias=B_t[ci],
                                             scale=A_t[ci])

        # streaming pool is closed; attention-phase pools may now reuse its
        # SBUF range
        kqp = ctx.enter_context(tc.tile_pool(name="kqp", bufs=1))
        attp = ctx.enter_context(tc.tile_pool(name="attp", bufs=1))
        outp = ctx.enter_context(tc.tile_pool(name="outp", bufs=1))

        # ---- projections ----
        # vT[j] : 32 tiles of [128 (j), 512 (c)], bias row via K=1 matmul
        vt_sb = []
        for jt in range(NJ):
            ps = psum.tile([P, C], F32, tag="mm", name=f"vps{jt}")
            for ci in range(NCH):
                nc.tensor.matmul(ps[:], lhsT=h_sb[ci][:, jt * P:(jt + 1) * P],
                                 rhs=wt["v"][ci][:], start=(ci == 0),
                                 stop=(not with_vbias and ci == NCH - 1))
            if with_vbias:
                nc.tensor.matmul(ps[:], lhsT=ones_row[:], rhs=bvr_sb[:],
                                 start=False, stop=True)
            vtt = kqp.tile([P, C], BF16, tag="vt", bufs=NJ, name=f"vt{jt}")
            nc.vector.tensor_copy(vtt[:], ps[:])
            vt_sb.append(vtt)
        # k[co, j] : 4 chunks of [128, 4096]
        k_sb = []
        for co in range(NCH):
            t = kqp.tile([P, NSEQ], BF16, tag=f"k{co}", name=f"k{co}")
            k_sb.append(t)
        for co in range(NCH):
            for jt in range(NSEQ // 512):
                ps = psum.tile([P, 512], F32, tag="mm", name=f"kps{co}_{jt}")
                for ci in range(NCH):
                    nc.tensor.matmul(ps[:], lhsT=wt["k"][ci][:, co * P:(co + 1) * P],
                                     rhs=h_sb[ci][:, jt * 512:(jt + 1) * 512],
                                     start=(ci == 0), stop=(ci == NCH - 1))
                nc.scalar.activation(k_sb[co][:, jt * 512:(jt + 1) * 512], ps[:],
                                     AF.Identity, bias=bpk_sb[co][:, 1:2])
        # q[co, i] : 4 chunks of [128, 2048] (own query half = cols 0:2048)
        q_sb = []
        for co in range(NCH):
            t = kqp.tile([P, NQ], BF16, tag=f"q{co}", name=f"q{co}")
            q_sb.append(t)
        for it in range(NQ // 512):
            for co in range(NCH):
                ps = psum.tile([P, 512], F32, tag="mm", name=f"qps{co}_{it}")
                for ci in range(NCH):
                    nc.tensor.matmul(ps[:], lhsT=wt["q"][ci][:, co * P:(co + 1) * P],
                                     rhs=h_sb[ci][:, it * 512:(it + 1) * 512],
                                     start=(ci == 0), stop=(ci == NCH - 1))
                nc.scalar.activation(q_sb[co][:, it * 512:(it + 1) * 512], ps[:],
                                     AF.Identity, bias=bpk_sb[co][:, 0:1])

        # ---- attention + fused output projection ----
        for ic in range(NI):
            accs = [psum.tile([P, 512], F32, tag="acc", name=f"acc{ic}_{c}")
                    for c in range(NCH)]
            eacc_prev = None
            for jt in range(NJ):
                ps = psum.tile([P, 512], F32, tag="mm", name=f"sps{ic}_{jt}")
                for ci in range(NCH):
                    nc.tensor.matmul(ps[:], lhsT=k_sb[ci][:, jt * P:(jt + 1) * P],
                                     rhs=q_sb[ci][:, ic * 512:(ic + 1) * 512],
                                     start=(ci == 0), stop=(ci == NCH - 1))
                et = attp.tile([P, 512], BF16, tag="et", bufs=3,
                               name=f"et{ic}_{jt}")
                nc.scalar.activation(et[:], ps[:], AF.Exp, scale=SCALE)
                eacc = attp.tile([P, 512], F32, tag="ea", bufs=2,
                                 name=f"ea{ic}_{jt}")
                if jt == 0:
                    nc.vector.tensor_copy(eacc[:], et[:])
                else:
                    nc.vector.tensor_add(eacc[:], eacc_prev[:], et[:])
                eacc_prev = eacc
                for c in range(NCH):
                    nc.tensor.matmul(accs[c][:], lhsT=vt_sb[jt][:, c * P:(c + 1) * P],
                                     rhs=et[:], start=(jt == 0), stop=(jt == NJ - 1))
            h2c = []
            for c in range(NCH):
                h2t = attp.tile([P, 512], BF16, tag="h2", bufs=2 * NCH,
                                name=f"h2_{ic}_{c}")
                nc.scalar.copy(h2t[:], accs[c][:])
                h2c.append(h2t)
            rcs = []
            gate_inst = None
            for iq in range(4):
                dps = psum.tile([P, 1], F32, tag="mm", name=f"dps{ic}_{iq}")
                mm_i = nc.tensor.matmul(dps[:],
                                        lhsT=eacc_prev[:, iq * P:(iq + 1) * P],
                                        rhs=ones_col[:], start=True, stop=True)
                if iq == 0:
                    gate_inst = mm_i
                rc = consts.tile([P, 1], F32, tag=f"rc{ic * 4 + iq}",
                                 name=f"rc{ic * 4 + iq}")
                nc.vector.reciprocal(rc[:], dps[:])
                rcs.append(rc)
            # output projection for this i-chunk (transposed) + residual
            for iq in range(4):
                t_i = ic * 4 + iq
                pps = psum.tile([P, C], F32, tag="mm", name=f"pps{t_i}")
                for c in range(NCH):
                    nc.tensor.matmul(pps[:], lhsT=h2c[c][:, iq * P:(iq + 1) * P],
                                     rhs=wt["p"][c][:], start=(c == 0),
                                     stop=(c == NCH - 1))
                xt = outp.tile([P, C], F32, tag="xr", bufs=3, name=f"xt{t_i}")
                xt_dma = nc.sync.dma_start(xt[:], xpbT_d[t_i * P:(t_i + 1) * P, :])
                # keep the residual loads out of the phase-A DMA window: only
                # issue them once this i-chunk's attention is winding down
                add_dep_helper(xt_dma.ins, gate_inst.ins, sync=True,
                               reason="delay residual load")
                ot = outp.tile([P, C], F32, tag="ot", bufs=3, name=f"ot{t_i}")
                nc.vector.scalar_tensor_tensor(ot[:], in0=pps[:],
                                               scalar=rcs[iq][:], in1=xt[:],
                                               op0=ALU.mult, op1=ALU.add)
                nc.sync.dma_start(out_d[t_i * P:(t_i + 1) * P, :], ot[:])

    nc.compile()
    if not nc.is_finalized():
        nc.finalize()
    return nc


_NC_CACHE = {}


def _get_nc(with_vbias=True):
    if with_vbias not in _NC_CACHE:
        _NC_CACHE[with_vbias] = build_nc(with_vbias)
    return _NC_CACHE[with_vbias]


def make_in_maps(x, gn_w, gn_b, wq, bq, wk, bk, wv, bv, wp, bp):
    bf = ml_dtypes.bfloat16
    x = np.asarray(x, np.float32)
    B = x.shape[0]
    shared = {
        "wqT": np.ascontiguousarray(np.asarray(wq, np.float32).T).astype(bf),
        "wkT": np.ascontiguousarray(np.asarray(wk, np.float32).T).astype(bf),
        "wvT": np.ascontiguousarray(np.asarray(wv, np.float32).T).astype(bf),
        "wpT": np.ascontiguousarray(np.asarray(wp, np.float32).T).astype(bf),
        "bpk": np.ascontiguousarray(
            np.stack([bq, bk, gn_w, gn_b], axis=1).astype(np.float32)),
        "bvr": np.asarray(bv, np.float32).reshape(1, C).astype(bf),
        "gmat": np.kron(np.eye(8, dtype=np.float32),
                        np.ones((16, 16), np.float32)),
    }
    in_maps = []
    for core in range(2 * B):
        b, h = divmod(core, 2)
        xb2 = x[b].reshape(C, NSEQ)
        own = xb2[:, h * NQ:(h + 1) * NQ]
        other = xb2[:, (1 - h) * NQ:(2 - h) * NQ]
        m = dict(shared)
        m["x"] = np.ascontiguousarray(
            np.concatenate([own, other], axis=1)).astype(bf)
        m["xpbT"] = np.ascontiguousarray(own.T + np.asarray(bp, np.float32)[None, :])
        in_maps.append(m)
    return in_maps


def kernel(x, gn_w, gn_b, wq, bq, wk, bk, wv, bv, wp, bp, _run_kwargs=None):
    x = np.asarray(x)
    B, C_, H, W = x.shape
    with_vbias = bool(np.any(np.asarray(bv, np.float32)))
    nc = _get_nc(with_vbias)
    in_maps = make_in_maps(x, gn_w, gn_b, wq, bq, wk, bk, wv, bv, wp, bp)
    res = run_bass_kernel_spmd(nc, in_maps, list(range(2 * B)),
                               **(_run_kwargs or {}))
    out = np.empty((B, C, NSEQ), np.float32)
    for core in range(2 * B):
        b, h = divmod(core, 2)
        out[b][:, h * NQ:(h + 1) * NQ] = res.results[core]["outT"].T
    out = out.reshape(B, C, H, W).astype(x.dtype, copy=False)
    kernel.last_results = res
    return out
